# revision 14
# baseline (speedup 1.0000x reference)
"""Trainium2 Bass kernel for nn_NeuralNetwork_31447750541324.

Network: per-frame conv stack (stride==kernel convs -> pure matmuls) ->
BatchNorm1d over (B, len) -> per-sample channel reorder by range ->
3 Elman RNNs (input 1, hidden 256) over F=64 steps -> mean -> linear.

Sharding: launch A runs the conv stack data-parallel over the 640 frames
(80 frames/core on 8 cores).  The tiny [640,3] conv result is re-arranged
on host (BN stats + affine, range argsort, channel select: ~10k FLOPs),
then launch B runs the 3 RNNs on 3 cores (one RNN each) including the
final linear projection; host sums the 3 partial projections + bias.

v2: fixed tile rings instead of per-iteration pool tiles (shrinks the
tile-teardown postamble), unpadded 243-row conv1 DMA (two tensors,
128+115 rows, K=115 second matmul chunk), block-diagonal conv2 weights
(M=6 packs even/odd frame outputs -> half the conv2 instructions), conv2
interleaved in 4 chunks so only 1/4 runs in the post-stream tail, and
RNN input DMAs spread across 4 queues.
"""

import os
import numpy as np

# ---------------- static problem dims ----------------
B, F, C, H, W = 10, 64, 3, 180, 180
NF = B * F                      # 640 frames
NCORES = 8
FPC = NF // NCORES              # 80 frames per core
CH, OUT, NCLS = 64, 256, 5
K1, K2 = 9, 9                   # conv1 kernel (9x9, stride 9)
KC1 = C * 9 * 9                 # 243 contraction
KCA, KCB = 128, KC1 - 128       # 128 + 115 row chunks
N1 = 400                        # 20x20 conv1 output positions
EPS = 1e-5

_cache = {}


def _f32(a):
    return np.ascontiguousarray(a, dtype=np.float32)


# ---------------- launch A: conv stack, 8 cores ----------------
# 8-frame DMA groups; 2 frames packed per PSUM tile via column-tiled
# matmuls (partitions 0-63 = even frame, 64-127 = odd frame); ACT does
# relu+bias from PSUM, DVE maxpools in fp16; conv2 in 4 chunks with
# block-diagonal weights (even-frame outs on rows 0-2, odd on 3-5).
GRP = 8           # frames per DMA group
NGRP = FPC // GRP
NPAIR = FPC // 2  # 40 psum pairs
NQ = 4            # conv2 chunks
NPQ = NPAIR // NQ  # 10 pairs per chunk
C2TRIG = {2: 0, 4: 1, 7: 2}  # conv2 chunk c after group g


def _build_conv_nc():
    import concourse.bacc as bacc
    import concourse.bass as bass
    import concourse.mybir as mybir
    import concourse.tile as tile

    f16, f32 = mybir.dt.float16, mybir.dt.float32
    nc = bacc.Bacc("TRN2", target_bir_lowering=False, debug=False,
                   num_devices=NCORES)

    pa = nc.dram_tensor("pa", [NGRP, 128, GRP, N1], f16,
                        kind="ExternalInput")
    pb1 = nc.dram_tensor("pb1", [NGRP, 112, GRP, N1], f16,
                         kind="ExternalInput")
    pb2 = nc.dram_tensor("pb2", [NGRP, 3, GRP, N1], f16,
                         kind="ExternalInput")
    w1 = nc.dram_tensor("w1", [128, 2, 128], f16, kind="ExternalInput")
    w2 = nc.dram_tensor("w2", [128, 25, 6], f16, kind="ExternalInput")
    b1 = nc.dram_tensor("b1", [128, 1], f32, kind="ExternalInput")
    b2 = nc.dram_tensor("b2", [6, 1], f32, kind="ExternalInput")
    yp = nc.dram_tensor("ypart", [6, NQ, NPQ], f32, kind="ExternalOutput")

    Relu = mybir.ActivationFunctionType.Relu
    X, XY = mybir.AxisListType.X, mybir.AxisListType.XY
    mx = mybir.AluOpType.max

    NFR = 10  # frame-tile ring depth: all groups resident, DMA never
    # throttles on a ring WAR (10 x 12.8KB/partition = 128KB SBUF)

    with tile.TileContext(nc) as tc:
        with (
            tc.tile_pool(name="const", bufs=1) as cp,
            tc.tile_pool(name="ps1", bufs=1, space=bass.MemorySpace.PSUM) as pp1,
            tc.tile_pool(name="ps2", bufs=1, space=bass.MemorySpace.PSUM) as pp2,
        ):
            w1s = cp.tile([128, 2, 128], f16, tag="w1")
            w2s = cp.tile([128, 25, 6], f16, tag="w2")
            b1s = cp.tile([128, 1], f32, tag="b1")
            b2s = cp.tile([6, 1], f32, tag="b2")
            pool1 = cp.tile([128, NPAIR, 100], f16, tag="pool1")
            yo = cp.tile([6, NQ, NPQ], f32, tag="yo")
            frt = [cp.tile([128, 2, GRP, N1], f16, tag=f"fr{i}",
                           name=f"fr{i}") for i in range(NFR)]
            pst = [pp1.tile([128, 100, 4], f32, tag=f"ps{i}",
                            name=f"ps{i}") for i in range(4)]
            rtt = [cp.tile([128, 100], f32, tag=f"rt{i}", name=f"rt{i}")
                   for i in range(4)]
            ps2t = [pp2.tile([6, NPQ, 2, 2], f32, tag=f"ps2{i}",
                             name=f"ps2{i}") for i in range(2)]
            rt2t = [cp.tile([6, NPQ], f32, tag=f"rt2{i}", name=f"rt2{i}")
                    for i in range(2)]
            # consts first on scalar's queue
            nc.scalar.dma_start(w1s[:], w1[:])
            nc.scalar.dma_start(b1s[:], b1[:])
            nc.scalar.dma_start(w2s[:], w2[:])
            nc.scalar.dma_start(b2s[:], b2[:])

            pv = pool1[:].rearrange("p q (a x b y) -> p q a x b y",
                                    a=2, x=5, b=2)

            def conv2_chunk(c):
                sl = slice(NPQ * c, NPQ * (c + 1))
                ps = ps2t[c % 2]
                for j in range(25):
                    kh, kw = j // 5, j % 5
                    nc.tensor.matmul(ps[:], w2s[:, j, :],
                                     pv[:, sl, :, kh, :, kw],
                                     start=(j == 0), stop=(j == 24))
                rt2 = rt2t[c % 2]
                nc.vector.tensor_reduce(rt2[:], ps[:], axis=XY, op=mx)
                nc.scalar.activation(yo[:, c, :], rt2[:], Relu, bias=b2s[:])

            for g in range(NGRP):
                gt = frt[g % NFR]
                # two HWDGE queues pipeline: per-queue the transfers run
                # one-at-a-time at full engine fan-out (128- and 112-row
                # transfers both spread over all 16 DMA engines), so
                # splitting across queues hides each queue's ramp gaps
                nc.sync.dma_start(gt[:, 0], pa[g])
                nc.scalar.dma_start(gt[0:112, 1], pb1[g])
                nc.sync.dma_start(gt[112:KCB, 1], pb2[g])
                for p in range(GRP // 2):
                    fa, fb = 2 * p, 2 * p + 1
                    pr = g * (GRP // 2) + p
                    ps = pst[pr % 4]
                    nc.tensor.matmul(ps[0:64], w1s[:, 0, 0:64],
                                     gt[:, 0, fa, :], start=True, stop=False)
                    nc.tensor.matmul(ps[64:128], w1s[:, 0, 64:128],
                                     gt[:, 0, fb, :], start=True, stop=False)
                    nc.tensor.matmul(ps[0:64], w1s[0:KCB, 1, 0:64],
                                     gt[0:KCB, 1, fa, :],
                                     start=False, stop=True)
                    nc.tensor.matmul(ps[64:128], w1s[0:KCB, 1, 64:128],
                                     gt[0:KCB, 1, fb, :],
                                     start=False, stop=True)
                    rt = rtt[pr % 4]
                    nc.vector.tensor_reduce(rt[:], ps[:], axis=X, op=mx)
                    nc.scalar.activation(pool1[:, pr, :],
                                         rt[:], Relu, bias=b1s[:])
                if g in C2TRIG:
                    conv2_chunk(C2TRIG[g])
            conv2_chunk(NQ - 1)
            nc.sync.dma_start(yp[:], yo[:])

    nc.compile()
    return nc


# ---------------- launch B: one RNN per core, 3 cores ----------------
# Raw bass (no TileContext): manual semaphores with cumulative targets.
# Skips the Tile framework's ~9us end-of-kernel semaphore-space teardown
# and its serial input-DMA startup.
def _build_rnn_nc():
    import concourse.bacc as bacc
    import concourse.mybir as mybir

    f16, f32 = mybir.dt.float16, mybir.dt.float32
    nc = bacc.Bacc("TRN2", target_bir_lowering=False, debug=False,
                   num_devices=3)

    xb = nc.dram_tensor("xb", [128, F, B], f16, kind="ExternalInput")
    wh = nc.dram_tensor("whht", [128, 2, 2, 128], f16, kind="ExternalInput")
    cf = nc.dram_tensor("cfw", [128, 2, 129], f16, kind="ExternalInput")
    wl = nc.dram_tensor("wl3", [128, 2, 5], f16, kind="ExternalInput")
    pr = nc.dram_tensor("pr", [B, NCLS], f32, kind="ExternalOutput")

    Tanh = mybir.ActivationFunctionType.Tanh

    xbs = nc.alloc_sbuf_tensor("xbs", [128, F, B], f16)
    whs = nc.alloc_sbuf_tensor("whs", [128, 2, 2, 128], f16)
    cfs = nc.alloc_sbuf_tensor("cfs", [128, 2, 129], f16)
    wls = nc.alloc_sbuf_tensor("wls", [128, 2, 5], f16)
    h0 = nc.alloc_sbuf_tensor("h0", [128, 2, B], f16)
    h1 = nc.alloc_sbuf_tensor("h1", [128, 2, B], f16)
    po = nc.alloc_sbuf_tensor("po", [B, NCLS], f32)
    # [128, 2, 512] f32: mc stride = one PSUM bank, so each m-chunk
    # accumulation group has its own bank; fixed ping-pong pair.
    psA = nc.alloc_psum_tensor("psA", [128, 2, 512], f32)
    psB = nc.alloc_psum_tensor("psB", [128, 2, 512], f32)
    psf = nc.alloc_psum_tensor("psf", [B, NCLS], f32)

    s_in = nc.alloc_semaphore("s_in")    # xb col 0 (+16) and cf (+16)
    s_xr = nc.alloc_semaphore("s_xr")    # xb cols 1.. (+16)
    s_w = nc.alloc_semaphore("s_w")      # wh (+16), wl (+16)
    s_pe = nc.alloc_semaphore("s_pe")    # +1 per completed psum group
    s_act = nc.alloc_semaphore("s_act")  # +1 per tanh
    s_out = nc.alloc_semaphore("s_out")  # copy (+1), out dma (+16)

    # parallel input DMAs on the two HWDGE queues; step 0 only needs
    # xb[:, 0] and cf, so those two go first on the fast 16-engine sync
    # queue; the rest of xb follows there (needed from step 1 on)
    nc.sync.dma_start(xbs[:, 0, :], xb[:, 0, :]).then_inc(s_in, 16)
    nc.sync.dma_start(cfs[:], cf[:]).then_inc(s_in, 16)
    nc.sync.dma_start(xbs[:, 1:, :], xb[:, 1:, :]).then_inc(s_xr, 16)
    nc.scalar.dma_start(whs[:], wh[:]).then_inc(s_w, 16)
    nc.scalar.dma_start(wls[:], wl[:]).then_inc(s_w, 16)

    hs, pss = [h0, h1], [psA, psB]
    zbias = cfs[:, 0, 128:129]           # all-zero [128,1] bias column

    # Software-pipelined: step t+1's input-injection matmuls (no h dep)
    # are emitted right after step t's recurrent matmuls, into the other
    # PSUM bank pair, so the PE executes them during step t's tanh.
    nc.tensor.wait_ge(s_in, 32)
    mm = None
    for mc in range(2):
        mm = nc.tensor.matmul(pss[0][:, mc, 0:B], cfs[:, mc, 0:128],
                              xbs[:, 0, :], start=True, stop=True)
    mm.then_inc(s_pe, 1)
    nc.tensor.wait_ge(s_xr, 16)          # rest of xb before t=1 injection
    nc.tensor.wait_ge(s_w, 16)           # whs ready before step 1
    for t in range(F):
        ps = pss[t % 2]
        if t > 0:
            h = hs[(t - 1) % 2]
            # s_act >= t: h(t-1) ready; also orders the injections below
            # behind ACT(t-1)'s read of pss[(t+1) % 2] (WAR)
            nc.tensor.wait_ge(s_act, t)
            for mc in range(2):
                nc.tensor.matmul(ps[:, mc, 0:B], whs[:, 0, mc, :],
                                 h[:, 0, :], start=False, stop=False)
                mm = nc.tensor.matmul(ps[:, mc, 0:B], whs[:, 1, mc, :],
                                      h[:, 1, :], start=False, stop=True)
            mm.then_inc(s_pe, 1)
        if t + 1 < F:
            nxt = pss[(t + 1) % 2]
            for mc in range(2):
                nc.tensor.matmul(nxt[:, mc, 0:B], cfs[:, mc, 0:128],
                                 xbs[:, t + 1, :], start=True, stop=False)
        nc.scalar.wait_ge(s_pe, t + 1)
        nc.scalar.activation(hs[t % 2][:], ps[:, :, 0:B], Tanh,
                             bias=zbias).then_inc(s_act, 1)

    h = hs[(F - 1) % 2]
    nc.tensor.wait_ge(s_w, 32)
    nc.tensor.wait_ge(s_act, F)
    nc.tensor.matmul(psf[:], h[:, 0, :], wls[:, 0, :],
                     start=True, stop=False)
    nc.tensor.matmul(psf[:], h[:, 1, :], wls[:, 1, :],
                     start=False, stop=True).then_inc(s_pe, 1)
    nc.vector.wait_ge(s_pe, F + 1)
    nc.vector.tensor_copy(po[:], psf[:]).then_inc(s_out, 1)
    nc.sync.wait_ge(s_out, 1)
    nc.sync.dma_start(pr[:], po[:]).then_inc(s_out, 16)
    # leave semaphores zeroed for the next execution of this NEFF
    nc.gpsimd.wait_ge(s_out, 17)
    for s in (s_in, s_xr, s_w, s_pe, s_act, s_out):
        nc.gpsimd.sem_clear(s)

    nc.compile()
    return nc


# ---------------- host-side input prep ----------------
def _prep_conv_inputs(x, W1, b1, W2, b2):
    # im2col for conv1: stride==kernel => non-overlapping patches.
    # n-order (oh10, ow10, ph, pw) groups each 2x2 maxpool window in the
    # last free axis; k-order (c, kh, kw) matches W1 flattening.
    xv = x.reshape(NF, C, 10, 2, 9, 20, 9)          # (fr,c,oh10,ph,kh,w,kw)
    xv = xv.reshape(NF, C, 10, 2, 9, 10, 2, 9)      # split w -> (ow10,pw)
    pat = xv.transpose(0, 1, 4, 7, 2, 5, 3, 6).reshape(NF, KC1, N1)
    pat = pat.astype(np.float16)
    pg = pat.reshape(NCORES, NGRP, GRP, KC1, N1).transpose(0, 1, 3, 2, 4)
    pa = np.ascontiguousarray(pg[:, :, 0:128])
    pb1 = np.ascontiguousarray(pg[:, :, 128:240])
    pb2 = np.ascontiguousarray(pg[:, :, 240:KC1])

    w1m = np.zeros((2 * 128, 64), np.float16)
    w1m[:KC1] = W1.reshape(64, KC1).T               # [K, M]
    w1c = w1m.reshape(2, 128, 64).transpose(1, 0, 2)  # [128, 2, 64]
    w1t = np.concatenate([w1c, w1c], axis=2)        # [128, 2, 128] dup cols
    w1t = np.ascontiguousarray(w1t)

    # conv2 lhsT per (kh,kw): block-diag [128, 6]; rows 0:64 (even-frame
    # channels) feed cols 0:3, rows 64:128 (odd-frame) feed cols 3:6
    w2c = W2.transpose(1, 2, 3, 0).reshape(64, 25, 3).astype(np.float16)
    w2t = np.zeros((128, 25, 6), np.float16)
    w2t[0:64, :, 0:3] = w2c
    w2t[64:128, :, 3:6] = w2c

    b1d = np.concatenate([b1, b1]).reshape(128, 1)
    b2d = np.concatenate([b2, b2]).reshape(6, 1)
    return pa, pb1, pb2, w1t, w2t, _f32(b1d), _f32(b2d)


def _prep_rnn_inputs(ts_r, Wih_r, Whh_r, bih_r, bhh_r, Wl):
    # ts_r: [F, B] f32 rank-r input sequence
    xbv = np.zeros((128, F, B), np.float16)
    xbv[0] = ts_r
    xbv[1] = 1.0
    wht = np.zeros((128, 2, 2, 128), np.float16)
    WhhT = Whh_r.T                                   # [k, m]
    for kc in range(2):
        for mc in range(2):
            wht[:, kc, mc, :] = WhhT[kc * 128:(kc + 1) * 128,
                                     mc * 128:(mc + 1) * 128]
    # col 128 stays zero: it doubles as the activation's zero-bias AP
    cfw = np.zeros((128, 2, 129), np.float16)
    bsum = bih_r + bhh_r
    for mc in range(2):
        cfw[0, mc, 0:128] = Wih_r[mc * 128:(mc + 1) * 128, 0]
        cfw[1, mc, 0:128] = bsum[mc * 128:(mc + 1) * 128]
    wl3 = np.zeros((128, 2, 5), np.float16)
    WlT3 = (Wl.T / 3.0)                              # [256, 5]
    for kc in range(2):
        wl3[:, kc, :] = WlT3[kc * 128:(kc + 1) * 128]
    return xbv, wht, cfw, wl3


def _ensure_profile_hook():
    """antenv.axon_hooks is absent in this image; synthesize it so
    run_bass_kernel_spmd(trace=True) can capture NTFF profiles."""
    import sys
    import types
    try:
        from antenv.axon_hooks import get_axon_ntff_profile_hook  # noqa
        return True
    except ImportError:
        pass
    try:
        sys.path.insert(0, "/root/.axon_site/trn_agent_boot")
        from trn_boot import _ntff_profile_via_ctypes
        hook = _ntff_profile_via_ctypes("/opt/axon/libaxon_pjrt.so")
        if hook is None:
            return False
        import antenv
        mod = types.ModuleType("antenv.axon_hooks")
        mod._hook = hook
        mod.get_axon_ntff_profile_hook = lambda: mod._hook
        mod.set_axon_ntff_profile_hook = lambda h: setattr(mod, "_hook", h)
        sys.modules["antenv.axon_hooks"] = mod
        antenv.axon_hooks = mod
        return True
    except Exception:
        return False


def _run(nc, in_maps, core_ids, label):
    from concourse.bass_utils import run_bass_kernel_spmd
    trace = os.environ.get("KERNEL_TRACE", "0") == "1"
    if trace:
        trace = _ensure_profile_hook()
    kw = {}
    if trace:
        import tempfile
        tdir = tempfile.mkdtemp(prefix=f"ktrace_{label}_")
        kw = {"tmpdir": tdir}
    res = run_bass_kernel_spmd(nc, in_maps, core_ids, trace=trace, **kw)
    _cache.setdefault("exec_ns", {})[label] = res.exec_time_ns
    _cache.setdefault("results_obj", {})[label] = res
    return res.results


# ---------------- main entry ----------------
def kernel(x, W1, b1, W2, b2, gamma, beta, Wih, Whh, bih, bhh, Wl, bl):
    x, W1, b1, W2, b2 = map(np.asarray, (x, W1, b1, W2, b2))
    gamma, beta = np.asarray(gamma), np.asarray(beta)
    Wih, Whh, bih, bhh = map(np.asarray, (Wih, Whh, bih, bhh))
    Wl, bl = np.asarray(Wl), np.asarray(bl)

    if "conv" not in _cache:
        _cache["conv"] = _build_conv_nc()
    if "rnn" not in _cache:
        _cache["rnn"] = _build_rnn_nc()

    # ---- launch A: conv stack over 640 frames on 8 cores ----
    pa, pb1, pb2, w1t, w2t, b1c, b2c = _prep_conv_inputs(x, W1, b1, W2, b2)
    in_maps = [
        {"pa": pa[k], "pb1": pb1[k], "pb2": pb2[k], "w1": w1t, "w2": w2t,
         "b1": b1c, "b2": b2c}
        for k in range(NCORES)
    ]
    res = _run(_cache["conv"], in_maps, list(range(NCORES)), "conv")
    # ypart [6, NQ, NPQ]: chunk c, pair i -> frames 2*(NPQ*c+i) + {0,1}
    y = np.empty((NF, 3), np.float32)
    for k, r in enumerate(res):
        ypk = r["ypart"]
        fr = np.empty((FPC, 3), np.float32)
        for c in range(NQ):
            pairs = c * NPQ + np.arange(NPQ)
            fr[2 * pairs] = ypk[0:3, c, :].T
            fr[2 * pairs + 1] = ypk[3:6, c, :].T
        y[k * FPC:(k + 1) * FPC] = fr
    y = y.reshape(B, F, 3)

    # ---- host glue: BN (train-mode) + per-sample channel reorder ----
    mean = y.mean(axis=(0, 2), keepdims=True)
    var = y.var(axis=(0, 2), keepdims=True)
    yn = (y - mean) / np.sqrt(var + EPS) * gamma[None, :, None] \
        + beta[None, :, None]
    t = yn.transpose(0, 2, 1)                        # [B, 3, F]
    rng = t.max(-1) - t.min(-1)
    perm = np.argsort(rng, axis=1, kind="stable")
    tsel = np.take_along_axis(t, perm[:, :, None], axis=1)  # [B, 3, F]

    # ---- launch B: 3 RNNs on 3 cores (+ scaled final linear) ----
    in_maps_b = []
    for r in range(3):
        ts_r = tsel[:, r, :].T                       # [F, B]
        xbv, wht, cfw, wl3 = _prep_rnn_inputs(
            ts_r, Wih[r], Whh[r], bih[r], bhh[r], Wl)
        in_maps_b.append({"xb": xbv, "whht": wht, "cfw": cfw, "wl3": wl3})
    res_b = _run(_cache["rnn"], in_maps_b, [0, 1, 2], "rnn")

    out = res_b[0]["pr"] + res_b[1]["pr"] + res_b[2]["pr"] + bl[None, :]
    return out.astype(np.float32)


# revision 15
# speedup vs baseline: 1.1453x; 1.1453x over previous
"""Trainium2 Bass kernel for nn_NeuralNetwork_31447750541324.

Network: per-frame conv stack (stride==kernel convs -> pure matmuls) ->
BatchNorm1d over (B, len) -> per-sample channel reorder by range ->
3 Elman RNNs (input 1, hidden 256) over F=64 steps -> mean -> linear.

Sharding: launch A runs the conv stack data-parallel over the 640 frames
(80 frames/core on 8 cores).  The tiny [640,3] conv result is re-arranged
on host (BN stats + affine, range argsort, channel select: ~10k FLOPs),
then launch B runs the 3 RNNs on 3 cores (one RNN each) including the
final linear projection; host sums the 3 partial projections + bias.

v2: fixed tile rings instead of per-iteration pool tiles (shrinks the
tile-teardown postamble), unpadded 243-row conv1 DMA (two tensors,
128+115 rows, K=115 second matmul chunk), block-diagonal conv2 weights
(M=6 packs even/odd frame outputs -> half the conv2 instructions), conv2
interleaved in 4 chunks so only 1/4 runs in the post-stream tail, and
RNN input DMAs spread across 4 queues.
"""

import os
import numpy as np

# ---------------- static problem dims ----------------
B, F, C, H, W = 10, 64, 3, 180, 180
NF = B * F                      # 640 frames
NCORES = 8
FPC = NF // NCORES              # 80 frames per core
CH, OUT, NCLS = 64, 256, 5
K1, K2 = 9, 9                   # conv1 kernel (9x9, stride 9)
KC1 = C * 9 * 9                 # 243 contraction
KCA, KCB = 128, KC1 - 128       # 128 + 115 row chunks
N1 = 400                        # 20x20 conv1 output positions
EPS = 1e-5

_cache = {}


def _f32(a):
    return np.ascontiguousarray(a, dtype=np.float32)


# ---------------- launch A: conv stack, 8 cores ----------------
# 8-frame DMA groups; 2 frames packed per PSUM tile via column-tiled
# matmuls (partitions 0-63 = even frame, 64-127 = odd frame); ACT does
# relu+bias from PSUM, DVE maxpools in fp16; conv2 in 4 chunks with
# block-diagonal weights (even-frame outs on rows 0-2, odd on 3-5).
GRP = 8           # frames per DMA group
NGRP = FPC // GRP
NPAIR = FPC // 2  # 40 psum pairs
NQ = 4            # conv2 chunks
NPQ = NPAIR // NQ  # 10 pairs per chunk
C2TRIG = {2: 0, 4: 1, 7: 2}  # conv2 chunk c after group g


def _build_conv_nc():
    import concourse.bacc as bacc
    import concourse.mybir as mybir

    f16, f32 = mybir.dt.float16, mybir.dt.float32
    nc = bacc.Bacc("TRN2", target_bir_lowering=False, debug=False,
                   num_devices=NCORES)

    pa = nc.dram_tensor("pa", [NGRP, 128, GRP, N1], f16,
                        kind="ExternalInput")
    pb1 = nc.dram_tensor("pb1", [NGRP, 112, GRP, N1], f16,
                         kind="ExternalInput")
    pb2 = nc.dram_tensor("pb2", [NGRP, 3, GRP, N1], f16,
                         kind="ExternalInput")
    w1 = nc.dram_tensor("w1", [128, 2, 128], f16, kind="ExternalInput")
    w2 = nc.dram_tensor("w2", [128, 25, 6], f16, kind="ExternalInput")
    b1 = nc.dram_tensor("b1", [128, 1], f32, kind="ExternalInput")
    b2 = nc.dram_tensor("b2", [6, 1], f32, kind="ExternalInput")
    yp = nc.dram_tensor("ypart", [6, NQ, NPQ], f32, kind="ExternalOutput")

    Relu = mybir.ActivationFunctionType.Relu
    X, XY = mybir.AxisListType.X, mybir.AxisListType.XY
    mx = mybir.AluOpType.max

    w1s = nc.alloc_sbuf_tensor("w1s", [128, 2, 128], f16)
    w2s = nc.alloc_sbuf_tensor("w2s", [128, 25, 6], f16)
    b1s = nc.alloc_sbuf_tensor("b1s", [128, 1], f32)
    b2s = nc.alloc_sbuf_tensor("b2s", [6, 1], f32)
    pool1 = nc.alloc_sbuf_tensor("pool1", [128, NPAIR, 100], f16)
    yo = nc.alloc_sbuf_tensor("yo", [6, NQ, NPQ], f32)
    frt = [nc.alloc_sbuf_tensor(f"fr{i}", [128, 2, GRP, N1], f16)
           for i in range(NGRP)]
    rtt = [nc.alloc_sbuf_tensor(f"rt{i}", [128, 100], f32) for i in range(4)]
    rt2t = [nc.alloc_sbuf_tensor(f"rt2{i}", [6, NPQ], f32) for i in range(2)]
    pst = [nc.alloc_psum_tensor(f"ps{i}", [128, 100, 4], f32)
           for i in range(4)]
    ps2t = [nc.alloc_psum_tensor(f"ps2{i}", [6, NPQ, 2, 2], f32)
            for i in range(2)]

    s_da = nc.alloc_semaphore("s_da")    # sync queue: pa+pb2, 32/group
    s_db = nc.alloc_semaphore("s_db")    # scalar queue: pb1, 16/group
    s_c = nc.alloc_semaphore("s_c")      # consts: w1/b1/w2/b2, 16 each
    s_pe = nc.alloc_semaphore("s_pe")    # +1 per completed psum group
    s_dve = nc.alloc_semaphore("s_dve")  # +1 per reduce
    s_act = nc.alloc_semaphore("s_act")  # +1 per relu/store
    s_out = nc.alloc_semaphore("s_out")

    # consts on scalar queue (in-order: w1 at 16, b1 at 32, w2 48, b2 64)
    nc.scalar.dma_start(w1s[:], w1[:]).then_inc(s_c, 16)
    nc.scalar.dma_start(b1s[:], b1[:]).then_inc(s_c, 16)
    nc.scalar.dma_start(w2s[:], w2[:]).then_inc(s_c, 16)
    nc.scalar.dma_start(b2s[:], b2[:]).then_inc(s_c, 16)
    # prefetch first two groups' pb1 ahead of the ACT stream
    nc.scalar.dma_start(frt[0][0:112, 1], pb1[0]).then_inc(s_db, 16)
    nc.scalar.dma_start(frt[1][0:112, 1], pb1[1]).then_inc(s_db, 16)

    pv = pool1.ap().rearrange("p q (a x b y) -> p q a x b y", a=2, x=5, b=2)

    pe_ct = [0]          # completed psum accumulation groups
    dve_ct = [0]         # reduces
    act_ct = [0]         # activations
    pair_pe = {}         # pair -> s_pe target
    pair_dve = {}
    pair_act = {}
    ch_pe = {}
    ch_dve = {}
    ch_act = {}

    def conv2_chunk(c):
        ps = ps2t[c % 2]
        if c == 0:
            nc.tensor.wait_ge(s_c, 48)          # w2 loaded
        nc.tensor.wait_ge(s_act, pair_act[NPQ * (c + 1) - 1])
        if c >= 2:
            nc.tensor.wait_ge(s_dve, ch_dve[c - 2])   # ps2 ring WAR
        mm = None
        for j in range(25):
            kh, kw = j // 5, j % 5
            mm = nc.tensor.matmul(ps.ap(), w2s[:, j, :],
                                  pv[:, NPQ * c:NPQ * (c + 1), :, kh, :, kw],
                                  start=(j == 0), stop=(j == 24))
        mm.then_inc(s_pe, 1)
        pe_ct[0] += 1
        ch_pe[c] = pe_ct[0]
        rt2 = rt2t[c % 2]
        nc.vector.wait_ge(s_pe, ch_pe[c])
        if c >= 2:
            nc.vector.wait_ge(s_act, ch_act[c - 2])   # rt2 ring WAR
        nc.vector.tensor_reduce(rt2.ap(), ps.ap(), axis=XY,
                                op=mx).then_inc(s_dve, 1)
        dve_ct[0] += 1
        ch_dve[c] = dve_ct[0]
        nc.scalar.wait_ge(s_dve, ch_dve[c])
        nc.scalar.activation(yo[:, c, :], rt2.ap(), Relu,
                             bias=b2s.ap()).then_inc(s_act, 1)
        act_ct[0] += 1
        ch_act[c] = act_ct[0]

    first_mm = True
    for g in range(NGRP):
        gt = frt[g]
        nc.sync.dma_start(gt[:, 0], pa[g]).then_inc(s_da, 16)
        nc.sync.dma_start(gt[112:KCB, 1], pb2[g]).then_inc(s_da, 16)
        if g + 2 < NGRP:   # pb1 prefetched 2 groups ahead on scalar
            nc.scalar.dma_start(frt[g + 2][0:112, 1],
                                pb1[g + 2]).then_inc(s_db, 16)
        nc.tensor.wait_ge(s_da, 32 * (g + 1))
        nc.tensor.wait_ge(s_db, 16 * (g + 1))
        if first_mm:
            nc.tensor.wait_ge(s_c, 16)           # w1 loaded
            first_mm = False
        for p in range(GRP // 2):
            fa, fb = 2 * p, 2 * p + 1
            pr = g * (GRP // 2) + p
            ps = pst[pr % 4]
            if pr >= 4:
                nc.tensor.wait_ge(s_dve, pair_dve[pr - 4])  # ps ring WAR
            nc.tensor.matmul(ps[0:64], w1s[:, 0, 0:64],
                             gt[:, 0, fa, :], start=True, stop=False)
            nc.tensor.matmul(ps[64:128], w1s[:, 0, 64:128],
                             gt[:, 0, fb, :], start=True, stop=False)
            nc.tensor.matmul(ps[0:64], w1s[0:KCB, 1, 0:64],
                             gt[0:KCB, 1, fa, :], start=False, stop=False)
            nc.tensor.matmul(ps[64:128], w1s[0:KCB, 1, 64:128],
                             gt[0:KCB, 1, fb, :], start=False,
                             stop=True).then_inc(s_pe, 2)
            pe_ct[0] += 2
            pair_pe[pr] = pe_ct[0]
            rt = rtt[pr % 4]
            nc.vector.wait_ge(s_pe, pair_pe[pr])
            if pr >= 4:
                nc.vector.wait_ge(s_act, pair_act[pr - 4])  # rt ring WAR
            nc.vector.tensor_reduce(rt.ap(), ps.ap(), axis=X,
                                    op=mx).then_inc(s_dve, 1)
            dve_ct[0] += 1
            pair_dve[pr] = dve_ct[0]
            nc.scalar.wait_ge(s_dve, pair_dve[pr])
            if pr == 0:
                nc.scalar.wait_ge(s_c, 32)       # b1 loaded
            nc.scalar.activation(pool1[:, pr, :], rt.ap(), Relu,
                                 bias=b1s.ap()).then_inc(s_act, 1)
            act_ct[0] += 1
            pair_act[pr] = act_ct[0]
        if g in C2TRIG:
            conv2_chunk(C2TRIG[g])
    conv2_chunk(NQ - 1)

    nc.sync.wait_ge(s_act, act_ct[0])
    nc.sync.dma_start(yp[:], yo.ap()).then_inc(s_out, 16)
    nc.gpsimd.wait_ge(s_out, 16)
    for s in (s_da, s_db, s_c, s_pe, s_dve, s_act, s_out):
        nc.gpsimd.sem_clear(s)

    nc.compile()
    return nc


# ---------------- launch B: one RNN per core, 3 cores ----------------
# Raw bass (no TileContext): manual semaphores with cumulative targets.
# Skips the Tile framework's ~9us end-of-kernel semaphore-space teardown
# and its serial input-DMA startup.
def _build_rnn_nc():
    import concourse.bacc as bacc
    import concourse.mybir as mybir

    f16, f32 = mybir.dt.float16, mybir.dt.float32
    nc = bacc.Bacc("TRN2", target_bir_lowering=False, debug=False,
                   num_devices=3)

    xb = nc.dram_tensor("xb", [128, F, B], f16, kind="ExternalInput")
    wh = nc.dram_tensor("whht", [128, 2, 2, 128], f16, kind="ExternalInput")
    cf = nc.dram_tensor("cfw", [128, 2, 129], f16, kind="ExternalInput")
    wl = nc.dram_tensor("wl3", [128, 2, 5], f16, kind="ExternalInput")
    pr = nc.dram_tensor("pr", [B, NCLS], f32, kind="ExternalOutput")

    Tanh = mybir.ActivationFunctionType.Tanh

    xbs = nc.alloc_sbuf_tensor("xbs", [128, F, B], f16)
    whs = nc.alloc_sbuf_tensor("whs", [128, 2, 2, 128], f16)
    cfs = nc.alloc_sbuf_tensor("cfs", [128, 2, 129], f16)
    wls = nc.alloc_sbuf_tensor("wls", [128, 2, 5], f16)
    h0 = nc.alloc_sbuf_tensor("h0", [128, 2, B], f16)
    h1 = nc.alloc_sbuf_tensor("h1", [128, 2, B], f16)
    po = nc.alloc_sbuf_tensor("po", [B, NCLS], f32)
    # [128, 2, 512] f32: mc stride = one PSUM bank, so each m-chunk
    # accumulation group has its own bank; fixed ping-pong pair.
    psA = nc.alloc_psum_tensor("psA", [128, 2, 512], f32)
    psB = nc.alloc_psum_tensor("psB", [128, 2, 512], f32)
    psf = nc.alloc_psum_tensor("psf", [B, NCLS], f32)

    s_in = nc.alloc_semaphore("s_in")    # xb col 0 (+16) and cf (+16)
    s_xr = nc.alloc_semaphore("s_xr")    # xb cols 1.. (+16)
    s_w = nc.alloc_semaphore("s_w")      # wh (+16), wl (+16)
    s_pe = nc.alloc_semaphore("s_pe")    # +1 per completed psum group
    s_act = nc.alloc_semaphore("s_act")  # +1 per tanh
    s_out = nc.alloc_semaphore("s_out")  # copy (+1), out dma (+16)

    # parallel input DMAs on the two HWDGE queues; step 0 only needs
    # xb[:, 0] and cf, so those two go first on the fast 16-engine sync
    # queue; the rest of xb follows there (needed from step 1 on)
    nc.sync.dma_start(xbs[:, 0, :], xb[:, 0, :]).then_inc(s_in, 16)
    nc.sync.dma_start(cfs[:], cf[:]).then_inc(s_in, 16)
    nc.sync.dma_start(xbs[:, 1:, :], xb[:, 1:, :]).then_inc(s_xr, 16)
    nc.scalar.dma_start(whs[:], wh[:]).then_inc(s_w, 16)
    nc.scalar.dma_start(wls[:], wl[:]).then_inc(s_w, 16)

    hs, pss = [h0, h1], [psA, psB]
    zbias = cfs[:, 0, 128:129]           # all-zero [128,1] bias column

    # Software-pipelined: step t+1's input-injection matmuls (no h dep)
    # are emitted right after step t's recurrent matmuls, into the other
    # PSUM bank pair, so the PE executes them during step t's tanh.
    nc.tensor.wait_ge(s_in, 32)
    mm = None
    for mc in range(2):
        mm = nc.tensor.matmul(pss[0][:, mc, 0:B], cfs[:, mc, 0:128],
                              xbs[:, 0, :], start=True, stop=True)
    mm.then_inc(s_pe, 1)
    nc.tensor.wait_ge(s_xr, 16)          # rest of xb before t=1 injection
    nc.tensor.wait_ge(s_w, 16)           # whs ready before step 1
    for t in range(F):
        ps = pss[t % 2]
        if t > 0:
            h = hs[(t - 1) % 2]
            # s_act >= t: h(t-1) ready; also orders the injections below
            # behind ACT(t-1)'s read of pss[(t+1) % 2] (WAR)
            nc.tensor.wait_ge(s_act, t)
            for mc in range(2):
                nc.tensor.matmul(ps[:, mc, 0:B], whs[:, 0, mc, :],
                                 h[:, 0, :], start=False, stop=False)
                mm = nc.tensor.matmul(ps[:, mc, 0:B], whs[:, 1, mc, :],
                                      h[:, 1, :], start=False, stop=True)
            mm.then_inc(s_pe, 1)
        if t + 1 < F:
            nxt = pss[(t + 1) % 2]
            for mc in range(2):
                nc.tensor.matmul(nxt[:, mc, 0:B], cfs[:, mc, 0:128],
                                 xbs[:, t + 1, :], start=True, stop=False)
        nc.scalar.wait_ge(s_pe, t + 1)
        nc.scalar.activation(hs[t % 2][:], ps[:, :, 0:B], Tanh,
                             bias=zbias).then_inc(s_act, 1)

    h = hs[(F - 1) % 2]
    nc.tensor.wait_ge(s_w, 32)
    nc.tensor.wait_ge(s_act, F)
    nc.tensor.matmul(psf[:], h[:, 0, :], wls[:, 0, :],
                     start=True, stop=False)
    nc.tensor.matmul(psf[:], h[:, 1, :], wls[:, 1, :],
                     start=False, stop=True).then_inc(s_pe, 1)
    nc.vector.wait_ge(s_pe, F + 1)
    nc.vector.tensor_copy(po[:], psf[:]).then_inc(s_out, 1)
    nc.sync.wait_ge(s_out, 1)
    nc.sync.dma_start(pr[:], po[:]).then_inc(s_out, 16)
    # leave semaphores zeroed for the next execution of this NEFF
    nc.gpsimd.wait_ge(s_out, 17)
    for s in (s_in, s_xr, s_w, s_pe, s_act, s_out):
        nc.gpsimd.sem_clear(s)

    nc.compile()
    return nc


# ---------------- host-side input prep ----------------
def _prep_conv_inputs(x, W1, b1, W2, b2):
    # im2col for conv1: stride==kernel => non-overlapping patches.
    # n-order (oh10, ow10, ph, pw) groups each 2x2 maxpool window in the
    # last free axis; k-order (c, kh, kw) matches W1 flattening.
    xv = x.reshape(NF, C, 10, 2, 9, 20, 9)          # (fr,c,oh10,ph,kh,w,kw)
    xv = xv.reshape(NF, C, 10, 2, 9, 10, 2, 9)      # split w -> (ow10,pw)
    pat = xv.transpose(0, 1, 4, 7, 2, 5, 3, 6).reshape(NF, KC1, N1)
    pat = pat.astype(np.float16)
    pg = pat.reshape(NCORES, NGRP, GRP, KC1, N1).transpose(0, 1, 3, 2, 4)
    pa = np.ascontiguousarray(pg[:, :, 0:128])
    pb1 = np.ascontiguousarray(pg[:, :, 128:240])
    pb2 = np.ascontiguousarray(pg[:, :, 240:KC1])

    w1m = np.zeros((2 * 128, 64), np.float16)
    w1m[:KC1] = W1.reshape(64, KC1).T               # [K, M]
    w1c = w1m.reshape(2, 128, 64).transpose(1, 0, 2)  # [128, 2, 64]
    w1t = np.concatenate([w1c, w1c], axis=2)        # [128, 2, 128] dup cols
    w1t = np.ascontiguousarray(w1t)

    # conv2 lhsT per (kh,kw): block-diag [128, 6]; rows 0:64 (even-frame
    # channels) feed cols 0:3, rows 64:128 (odd-frame) feed cols 3:6
    w2c = W2.transpose(1, 2, 3, 0).reshape(64, 25, 3).astype(np.float16)
    w2t = np.zeros((128, 25, 6), np.float16)
    w2t[0:64, :, 0:3] = w2c
    w2t[64:128, :, 3:6] = w2c

    b1d = np.concatenate([b1, b1]).reshape(128, 1)
    b2d = np.concatenate([b2, b2]).reshape(6, 1)
    return pa, pb1, pb2, w1t, w2t, _f32(b1d), _f32(b2d)


def _prep_rnn_inputs(ts_r, Wih_r, Whh_r, bih_r, bhh_r, Wl):
    # ts_r: [F, B] f32 rank-r input sequence
    xbv = np.zeros((128, F, B), np.float16)
    xbv[0] = ts_r
    xbv[1] = 1.0
    wht = np.zeros((128, 2, 2, 128), np.float16)
    WhhT = Whh_r.T                                   # [k, m]
    for kc in range(2):
        for mc in range(2):
            wht[:, kc, mc, :] = WhhT[kc * 128:(kc + 1) * 128,
                                     mc * 128:(mc + 1) * 128]
    # col 128 stays zero: it doubles as the activation's zero-bias AP
    cfw = np.zeros((128, 2, 129), np.float16)
    bsum = bih_r + bhh_r
    for mc in range(2):
        cfw[0, mc, 0:128] = Wih_r[mc * 128:(mc + 1) * 128, 0]
        cfw[1, mc, 0:128] = bsum[mc * 128:(mc + 1) * 128]
    wl3 = np.zeros((128, 2, 5), np.float16)
    WlT3 = (Wl.T / 3.0)                              # [256, 5]
    for kc in range(2):
        wl3[:, kc, :] = WlT3[kc * 128:(kc + 1) * 128]
    return xbv, wht, cfw, wl3


def _ensure_profile_hook():
    """antenv.axon_hooks is absent in this image; synthesize it so
    run_bass_kernel_spmd(trace=True) can capture NTFF profiles."""
    import sys
    import types
    try:
        from antenv.axon_hooks import get_axon_ntff_profile_hook  # noqa
        return True
    except ImportError:
        pass
    try:
        sys.path.insert(0, "/root/.axon_site/trn_agent_boot")
        from trn_boot import _ntff_profile_via_ctypes
        hook = _ntff_profile_via_ctypes("/opt/axon/libaxon_pjrt.so")
        if hook is None:
            return False
        import antenv
        mod = types.ModuleType("antenv.axon_hooks")
        mod._hook = hook
        mod.get_axon_ntff_profile_hook = lambda: mod._hook
        mod.set_axon_ntff_profile_hook = lambda h: setattr(mod, "_hook", h)
        sys.modules["antenv.axon_hooks"] = mod
        antenv.axon_hooks = mod
        return True
    except Exception:
        return False


def _run(nc, in_maps, core_ids, label):
    from concourse.bass_utils import run_bass_kernel_spmd
    trace = os.environ.get("KERNEL_TRACE", "0") == "1"
    if trace:
        trace = _ensure_profile_hook()
    kw = {}
    if trace:
        import tempfile
        tdir = tempfile.mkdtemp(prefix=f"ktrace_{label}_")
        kw = {"tmpdir": tdir}
    res = run_bass_kernel_spmd(nc, in_maps, core_ids, trace=trace, **kw)
    _cache.setdefault("exec_ns", {})[label] = res.exec_time_ns
    _cache.setdefault("results_obj", {})[label] = res
    return res.results


# ---------------- main entry ----------------
def kernel(x, W1, b1, W2, b2, gamma, beta, Wih, Whh, bih, bhh, Wl, bl):
    x, W1, b1, W2, b2 = map(np.asarray, (x, W1, b1, W2, b2))
    gamma, beta = np.asarray(gamma), np.asarray(beta)
    Wih, Whh, bih, bhh = map(np.asarray, (Wih, Whh, bih, bhh))
    Wl, bl = np.asarray(Wl), np.asarray(bl)

    if "conv" not in _cache:
        _cache["conv"] = _build_conv_nc()
    if "rnn" not in _cache:
        _cache["rnn"] = _build_rnn_nc()

    # ---- launch A: conv stack over 640 frames on 8 cores ----
    pa, pb1, pb2, w1t, w2t, b1c, b2c = _prep_conv_inputs(x, W1, b1, W2, b2)
    in_maps = [
        {"pa": pa[k], "pb1": pb1[k], "pb2": pb2[k], "w1": w1t, "w2": w2t,
         "b1": b1c, "b2": b2c}
        for k in range(NCORES)
    ]
    res = _run(_cache["conv"], in_maps, list(range(NCORES)), "conv")
    # ypart [6, NQ, NPQ]: chunk c, pair i -> frames 2*(NPQ*c+i) + {0,1}
    y = np.empty((NF, 3), np.float32)
    for k, r in enumerate(res):
        ypk = r["ypart"]
        fr = np.empty((FPC, 3), np.float32)
        for c in range(NQ):
            pairs = c * NPQ + np.arange(NPQ)
            fr[2 * pairs] = ypk[0:3, c, :].T
            fr[2 * pairs + 1] = ypk[3:6, c, :].T
        y[k * FPC:(k + 1) * FPC] = fr
    y = y.reshape(B, F, 3)

    # ---- host glue: BN (train-mode) + per-sample channel reorder ----
    mean = y.mean(axis=(0, 2), keepdims=True)
    var = y.var(axis=(0, 2), keepdims=True)
    yn = (y - mean) / np.sqrt(var + EPS) * gamma[None, :, None] \
        + beta[None, :, None]
    t = yn.transpose(0, 2, 1)                        # [B, 3, F]
    rng = t.max(-1) - t.min(-1)
    perm = np.argsort(rng, axis=1, kind="stable")
    tsel = np.take_along_axis(t, perm[:, :, None], axis=1)  # [B, 3, F]

    # ---- launch B: 3 RNNs on 3 cores (+ scaled final linear) ----
    in_maps_b = []
    for r in range(3):
        ts_r = tsel[:, r, :].T                       # [F, B]
        xbv, wht, cfw, wl3 = _prep_rnn_inputs(
            ts_r, Wih[r], Whh[r], bih[r], bhh[r], Wl)
        in_maps_b.append({"xb": xbv, "whht": wht, "cfw": cfw, "wl3": wl3})
    res_b = _run(_cache["rnn"], in_maps_b, [0, 1, 2], "rnn")

    out = res_b[0]["pr"] + res_b[1]["pr"] + res_b[2]["pr"] + bl[None, :]
    return out.astype(np.float32)
